# revision 19
# baseline (speedup 1.0000x reference)
"""Masked multi-head attention block (B=8, N=1024, D=768, H=12) on 8 NeuronCores.

Strategy: pure data-parallel over batch (1 batch element per core).  Per core,
one fully-fused software pipeline keeps the PE array busy (and its HAM clock
gate warm) end to end:

  prologue:  need-ordered chunked input DMA across the three DMA queues;
             first matmul issues ~6us in.
  v-phase:   v[n, e] = xT(lhsT) @ WvT, stored bf16 in per-head-pair blocks
             [v_even | ones64][ones64 | v_odd]: the 64 ones columns replicate
             the softmax denominator Z across 64 PSUM partitions for free.
  qk(pr):    q,k e-tiles of head-pair pr (f32r matmuls, cast to bf16).
             Pairs 0,1 run before the attention loop; pair pr+2 is emitted as
             PE *filler* inside attention pair pr, exactly absorbing the PE
             slack of the ACT-bound softmax loop (keeps HAM at 2.4 GHz).
  attention: per (pair, hi, t): ST = kT(lhsT) @ qT (bf16, 1024-wide moving),
             P = exp(ST*scale + key_mask_bias) on ACT (PSUM -> SBUF bf16),
             diag add (padded queries -> one-hot), P@Vaug accumulated over t
             (PV on one 64-partition half, Z replicated on the other), then
             norm: recip(Z-half) -> tiny partition-shift DMA -> mul into otn.
  proj:      out[n, e] = otn(lhsT) @ WprojT + ones(K=1) x bproj, accumulated
             in PSUM and DMA'd straight PSUM -> DRAM (no SBUF staging).

Numerics: QKV/proj contractions (K=768) in float32r; q/k/v/P in bf16 (errors
~1e-3 absolute vs a ~6.0 absmax reference; tolerance is 2e-2 relative).
"""
import sys
for _p in ('/opt/trn_rl_repo',):
    if _p not in sys.path:
        sys.path.insert(0, _p)

from contextlib import ExitStack

import numpy as np

import concourse.bass as bass
import concourse.bacc as bacc
import concourse.mybir as mybir
import concourse.tile as tile
from concourse import bass_utils

F32 = mybir.dt.float32
F32R = mybir.dt.float32r
BF16 = mybir.dt.bfloat16
AF = mybir.ActivationFunctionType

B, N, D, H, HD = 8, 1024, 768, 12, 64
P = 128
DT = D // P            # 6 d-tiles
PAIRS = H // 2         # 6 head pairs (== DT)
SCALE = HD ** -0.5
NEGMASK = -30000.0     # exp(x + NEGMASK) == 0.0 in fp32 for any realistic score
BIGG = 1e15            # diagonal dominance constant for padded-query rows


def build_nc(n=N, debug=False):
    NT = n // P                    # seq tiles (8)
    nc = bacc.Bacc("TRN2", target_bir_lowering=False, debug=False)

    # xT is packed t-major ([P, t, d, 128]) and wqkT pair-major
    # ([P, pair, q/k, d, 128]) so every DMA chunk is contiguous per partition.
    xT_d = nc.dram_tensor("xT", [P, DT * n], F32, kind="ExternalInput")
    wqkT_d = nc.dram_tensor("wqkT", [P, DT * 2 * D], F32, kind="ExternalInput")
    wvT_d = nc.dram_tensor("wvT", [P, DT * D], F32, kind="ExternalInput")
    wprojT_d = nc.dram_tensor("wprojT", [P, DT * D], F32, kind="ExternalInput")
    bproj_d = nc.dram_tensor("bproj", [1, D], F32, kind="ExternalInput")
    mbias_d = nc.dram_tensor("mbias", [P, NT], F32, kind="ExternalInput")
    omm_d = nc.dram_tensor("omm", [P, NT], F32, kind="ExternalInput")
    ones_d = nc.dram_tensor("onesv", [1, 512], F32, kind="ExternalInput")
    out_d = nc.dram_tensor("out", [D, n], F32, kind="ExternalOutput")

    def rr(ap):
        return ap.bitcast(F32R)

    with tile.TileContext(nc) as tc, ExitStack() as ctx:
        persist = ctx.enter_context(tc.tile_pool(name="persist", bufs=1))
        xt = persist.tile([P, NT, DT, P], F32R)        # 24KB/part, t-major
        wq = persist.tile([P, PAIRS, 2, DT, P], F32R)  # 36KB, pair-major
        wpj = persist.tile([P, DT, D], F32R)           # 18KB
        vaug = persist.tile([P, NT, PAIRS, 2 * P], BF16)  # 24KB
        qks = persist.tile([P, 3, 2, n], BF16)         # 12KB (pair slots: q,k)
        otn = persist.tile([P, DT, n], F32R)           # 24KB
        pa = persist.tile([P, NT, n], BF16)            # 16KB
        pb = persist.tile([P, NT, n], BF16)            # 16KB
        dtl = persist.tile([P, NT, P], BF16)           # 2KB
        mb = persist.tile([P, NT], F32)
        om = persist.tile([P, NT], F32)
        ones = persist.tile([1, 512], F32R)
        bpj = persist.tile([1, D], F32R)

        # ---------------- input DMAs, need-ordered across 3 queues ---------
        xt_src = rr(xT_d.ap()).rearrange("p (t d c) -> p t d c", t=NT, d=DT)
        wqk_src = rr(wqkT_d.ap()).rearrange("p (pr j d c) -> p pr j d c",
                                            pr=PAIRS, j=2, d=DT)
        wv_src = rr(wvT_d.ap()).rearrange("p (dt e) -> p dt e", dt=DT)
        wpj_src = rr(wprojT_d.ap()).rearrange("p (dt e) -> p dt e", dt=DT)

        # gpsimd queue: small tensors, then the (late-needed) proj weights
        nc.gpsimd.dma_start(mb, mbias_d.ap())
        nc.gpsimd.dma_start(om, omm_d.ap())
        nc.gpsimd.dma_start(ones, rr(ones_d.ap()))
        nc.gpsimd.dma_start(bpj, rr(bproj_d.ap()))
        nc.gpsimd.dma_start(wpj, wpj_src)

        # sync queue: x by seq-tile (v-phase consumes in t order), then
        # q/k weights for pairs 2..5 (filler work, needed progressively)
        for t in range(NT):
            nc.sync.dma_start(xt[:, t], xt_src[:, t])
        for pr in range(2, PAIRS):
            nc.sync.dma_start(wq[:, pr], wqk_src[:, pr])

        # scalar queue: v weights (needed first), then q/k for pairs 0,1
        with tc.tile_pool(name="wvpool", bufs=1) as wvp, \
             tc.tile_pool(name="scratch", bufs=1) as scr:
            wv = wvp.tile([P, DT, D], F32R)            # 18KB, freed after v
            dtf = scr.tile([P, NT, P], F32)            # 4KB, prologue only
            warm = scr.tile([1, 1], F32)
            # fine-grained wv chunks so v-phase d-loop starts ASAP
            for d in range(DT):
                nc.scalar.dma_start(wv[:, d, 0:512], wv_src[:, d, 0:512])
            for d in range(DT):
                nc.scalar.dma_start(wv[:, d, 512:D], wv_src[:, d, 512:D])
            for pr in range(2):
                nc.scalar.dma_start(wq[:, pr], wqk_src[:, pr])

            # prologue compute: ones blocks of vaug, diag tiles, ACT warmup
            nc.vector.memset(vaug[:, :, :, HD:3 * HD], 1.0)
            for t in range(NT):
                nc.gpsimd.affine_select(
                    out=dtf[:, t, :],
                    in_=om[:, t:t + 1].to_broadcast((P, P)),
                    pattern=[[-1, P]],
                    compare_op=mybir.AluOpType.is_equal,
                    fill=0.0, base=0, channel_multiplier=1,
                )
            nc.vector.tensor_copy(dtl, dtf)
            # load the exp table set during the v-phase, not at first score
            nc.scalar.activation(warm, mb[0:1, 0:1], AF.Exp, bias=0.0, scale=1.0)

            with tc.tile_pool(name="stps", bufs=2, space="PSUM") as stps, \
                 tc.tile_pool(name="otps", bufs=2, space="PSUM") as otps, \
                 tc.tile_pool(name="rbsp", bufs=2) as rbsp:

                # ---------------- v-phase ----------------
                # per t: 12 accumulating MMs into one [P, 768] PSUM region,
                # then strided copies into vaug's pair blocks (even heads at
                # block cols 0:64, odd heads at 192:256).
                for t in range(NT):
                    vp = otps.tile([P, n], F32, tag="ot", name="vp")
                    for (cb, cw) in ((0, 512), (512, 256)):
                        for d in range(DT):
                            nc.tensor.matmul(
                                vp[:, cb:cb + cw],
                                xt[:, t, d, :],
                                wv[:, d, cb:cb + cw],
                                start=(d == 0), stop=(d == DT - 1))
                    vsp = vp[:, 0:D].rearrange("p (pr h d) -> p pr h d",
                                               h=2, d=HD)
                    # evens on ScE, odds on DVE (both engines idle here)
                    nc.scalar.copy(vaug[:, t, :, 0:HD], vsp[:, :, 0, :])
                    nc.vector.tensor_copy(vaug[:, t, :, 3 * HD:4 * HD],
                                          vsp[:, :, 1, :])

                # ---------------- qk units + attention pipeline ----------
                def emit_qk_quarter(pr, j, c):
                    """One c-half of q (j=0) or k (j=1) for pair pr: 6 MMs."""
                    key = (pr, j)
                    if key not in qk_ps:
                        qk_ps[key] = stps.tile([P, n], F32, tag="st",
                                               name="qkp")
                    ps = qk_ps[key]
                    for d in range(DT):
                        nc.tensor.matmul(
                            ps[:, c * 512:(c + 1) * 512],
                            wq[:, pr, j, d, :],
                            xt[:, 4 * c:4 * c + 4, d, :],
                            start=(d == 0), stop=(d == DT - 1))
                    if c == 1:
                        nc.vector.tensor_copy(qks[:, pr % 3, j, :], ps)
                        del qk_ps[key]

                qk_ps = {}
                for pr in range(2):
                    for j in range(2):
                        for c in range(2):
                            emit_qk_quarter(pr, j, c)

                ot_tiles = {}

                def emit_st(pr, hi, t):
                    slot = pr % 3
                    lo = hi * HD
                    st = stps.tile([P, n], F32, tag="st", name="st")
                    for c in range(2):
                        sl = slice(c * 512, (c + 1) * 512)
                        nc.tensor.matmul(
                            st[:, sl],
                            qks[lo:lo + HD, slot, 1, t * P:(t + 1) * P],
                            qks[lo:lo + HD, slot, 0, sl],
                            start=True, stop=True)
                    px = pa if hi == 0 else pb
                    nc.scalar.activation(px[:, t, :], st, AF.Exp,
                                         bias=mb[:, t:t + 1], scale=SCALE)
                    nc.vector.tensor_add(px[:, t, t * P:(t + 1) * P],
                                         px[:, t, t * P:(t + 1) * P],
                                         dtl[:, t, :])

                def emit_pv(pr, hi, t):
                    if (pr, hi) not in ot_tiles:
                        ot_tiles[(pr, hi)] = otps.tile([P, n], F32, tag="ot",
                                                       name="ot")
                    ot = ot_tiles[(pr, hi)]
                    px = pa if hi == 0 else pb
                    for c in range(2):
                        sl = slice(c * 512, (c + 1) * 512)
                        nc.tensor.matmul(
                            ot[:, sl], vaug[:, t, pr, hi * P:(hi + 1) * P],
                            px[:, t, sl],
                            start=(t == 0), stop=(t == NT - 1),
                            skip_group_check=True)

                def emit_norm(pr, hi):
                    # reciprocal_approx_fast only works at partition base 0
                    # (custom-DVE op); stage Z to base 0 where needed.
                    ot = ot_tiles.pop((pr, hi))
                    if hi == 0:      # PV on 0:64, Z replicated on 64:128
                        zst = rbsp.tile([P, n], F32, tag="rbs", name="zst")
                        rbs = rbsp.tile([P, n], F32, tag="rbs", name="rbs")
                        nc.vector.tensor_copy(zst[HD:P, :], ot[HD:P, :])
                        nc.gpsimd.dma_start(zst[0:HD, :], zst[HD:P, :])
                        nc.vector.reciprocal_approx_fast(rbs[0:HD, :],
                                                         zst[0:HD, :])
                        nc.vector.tensor_mul(otn[0:HD, pr, :], ot[0:HD, :],
                                             rbs[0:HD, :])
                    else:            # Z replicated on 0:64, PV on 64:128
                        rbs = rbsp.tile([P, n], F32, tag="rbs", name="rbs")
                        nc.vector.reciprocal_approx_fast(rbs[0:HD, :],
                                                         ot[0:HD, :])
                        nc.gpsimd.dma_start(rbs[HD:P, :], rbs[0:HD, :])
                        nc.vector.tensor_mul(otn[HD:P, pr, :], ot[HD:P, :],
                                             rbs[HD:P, :])

                groups = [(pr, hi, t)
                          for pr in range(PAIRS)
                          for hi in range(2) for t in range(NT)]
                LAG = 2
                for i, g in enumerate(groups):
                    emit_st(*g)
                    pr, hi, t = g
                    # filler: qk projection for pair pr+2 (absorbs PE slack;
                    # keeps HAM warm).  Quarter-units at adjacent groups so a
                    # held accumulation spans at most one 'st' ring slot.
                    if pr + 2 < PAIRS and t in (2, 3):
                        emit_qk_quarter(pr + 2, hi, t - 2)
                    if i >= LAG:
                        gj = groups[i - LAG]
                        emit_pv(*gj)
                        if gj[2] == NT - 1:
                            emit_norm(gj[0], gj[1])
                for i in range(len(groups) - LAG, len(groups)):
                    gj = groups[i]
                    emit_pv(*gj)
                    if gj[2] == NT - 1:
                        emit_norm(gj[0], gj[1])
                # bridge matmuls: keep the PE (and its HAM clock) busy while
                # the last norm chain runs, so the projection starts warm.
                # Results are never read.
                for c in range(2):
                    brg = stps.tile([P, n], F32, tag="st", name="brg")
                    for d in range(DT):
                        nc.tensor.matmul(
                            brg[:, c * 512:(c + 1) * 512],
                            wq[:, 0, 0, d, :], xt[:, 4 * c:4 * c + 4, d, :],
                            start=(d == 0), stop=(d == DT - 1))

        # ---------------- output projection (transposed: outT[e, n]) -------
        # wproj blocks are the stationary operand (6x fewer LDWEIGHTS than
        # otn-stationary); the host transposes the [D, n] result back.
        with tc.tile_pool(name="pjp", bufs=3, space="PSUM") as pjp, \
             tc.tile_pool(name="obp", bufs=3) as obp:
            for et in range(DT):
                ps = pjp.tile([P, n], F32, tag="pj")
                ob = obp.tile([P, n], F32, tag="ob")
                for c in range(2):
                    sl = slice(c * 512, (c + 1) * 512)
                    # bias first: it has no dependency on the attention
                    # output, so it issues during the final norm chain
                    nc.tensor.matmul(ps[:, sl],
                                     bpj[:, et * P:(et + 1) * P],
                                     ones, start=True, stop=False)
                    for d in range(DT):
                        nc.tensor.matmul(ps[:, sl],
                                         wpj[:, d, et * P:(et + 1) * P],
                                         otn[:, d, sl],
                                         start=False, stop=(d == DT - 1))
                # ACT and DVE are both idle post-attention: split the drain
                nc.scalar.copy(ob[:, 0:512], ps[:, 0:512])
                nc.vector.tensor_copy(ob[:, 512:n], ps[:, 512:n])
                (nc.sync if et % 2 == 0 else nc.scalar).dma_start(
                    out_d.ap()[et * P:(et + 1) * P, :], ob)

    nc.compile()
    return nc


def make_in_maps(x, mask, Wqkv, Wproj, bproj):
    x = np.ascontiguousarray(np.asarray(x), dtype=np.float32)
    mask = np.asarray(mask)
    def pack(wt):   # [D, cols] -> [128, DT*cols], row p = concat_d wt[d*128+p]
        cols = wt.shape[1]
        return np.ascontiguousarray(
            wt.reshape(DT, P, cols).transpose(1, 0, 2).reshape(P, DT * cols))
    wqkvT = np.asarray(Wqkv, dtype=np.float32).T.copy()   # [D, 3D]
    # pair-major q/k pack: [P, pair, q/k, d, 128]
    wqkT = np.ascontiguousarray(
        wqkvT[:, 0:2 * D].reshape(DT, P, 2, PAIRS, P)
        .transpose(1, 3, 2, 0, 4).reshape(P, DT * 2 * D))
    wvT = pack(wqkvT[:, 2 * D:3 * D])
    wprojT = pack(np.asarray(Wproj, dtype=np.float32).T.copy())
    bp = np.ascontiguousarray(np.asarray(bproj, dtype=np.float32).reshape(1, D))
    onesv = np.ones((1, 512), dtype=np.float32)
    b, n, _ = x.shape
    nt = n // P
    def pack_x_tmajor(xTi):   # [D, n] -> [P, t, d, 128] flattened
        return np.ascontiguousarray(
            xTi.reshape(DT, P, nt, P).transpose(1, 2, 0, 3)
            .reshape(P, DT * n))
    in_maps = []
    for i in range(b):
        mf = mask[i].astype(np.float32)
        mcol = mf.reshape(nt, P).T.copy()              # [P, NT]
        in_maps.append({
            "xT": pack_x_tmajor(np.ascontiguousarray(x[i].T)),
            "wqkT": wqkT,
            "wvT": wvT,
            "wprojT": wprojT,
            "bproj": bp,
            "mbias": np.ascontiguousarray((mcol - 1.0) * (-NEGMASK)),
            "omm": np.ascontiguousarray((1.0 - mcol) * BIGG),
            "onesv": onesv,
        })
    return in_maps


_NC_CACHE = {}


def get_nc(n=N):
    if n not in _NC_CACHE:
        _NC_CACHE[n] = build_nc(n)
    return _NC_CACHE[n]


def kernel(x, mask, Wqkv, Wproj, bproj):
    x = np.asarray(x)
    b, n, _ = x.shape
    nc = get_nc(n)
    in_maps = make_in_maps(x, mask, Wqkv, Wproj, bproj)
    res = bass_utils.run_bass_kernel_spmd(nc, in_maps, core_ids=list(range(b)))
    out = np.stack([np.asarray(res.results[i]["out"]).T for i in range(b)],
                   axis=0)
    return np.ascontiguousarray(out).astype(np.float32)


# revision 27
# speedup vs baseline: 1.0463x; 1.0463x over previous
"""Masked multi-head attention block (B=8, N=1024, D=768, H=12) on 8 NeuronCores.

Strategy: pure data-parallel over batch (1 batch element per core).  Per core,
one fully-fused software pipeline keeps the PE array busy (and its HAM clock
gate warm) end to end:

  prologue:  need-ordered chunked input DMA across the three DMA queues;
             first matmul issues ~6us in.
  v-phase:   v[n, e] = xT(lhsT) @ WvT, stored bf16 in per-head-pair blocks
             [v_even | ones64][ones64 | v_odd]: the 64 ones columns replicate
             the softmax denominator Z across 64 PSUM partitions for free.
  qk(pr):    q,k e-tiles of head-pair pr (f32r matmuls, cast to bf16).
             Pairs 0,1 run before the attention loop; pair pr+2 is emitted as
             PE *filler* inside attention pair pr, exactly absorbing the PE
             slack of the ACT-bound softmax loop (keeps HAM at 2.4 GHz).
  attention: per (pair, hi, t): ST = kT(lhsT) @ qT (bf16, 1024-wide moving),
             P = exp(ST*scale + key_mask_bias) on ACT (PSUM -> SBUF bf16),
             diag add (padded queries -> one-hot), P@Vaug accumulated over t
             (PV on one 64-partition half, Z replicated on the other), then
             norm: recip(Z-half) -> tiny partition-shift DMA -> mul into otn.
  proj:      out[n, e] = otn(lhsT) @ WprojT + ones(K=1) x bproj, accumulated
             in PSUM and DMA'd straight PSUM -> DRAM (no SBUF staging).

Numerics: QKV/proj contractions (K=768) in float32r; q/k/v/P in bf16 (errors
~1e-3 absolute vs a ~6.0 absmax reference; tolerance is 2e-2 relative).
"""
import sys
for _p in ('/opt/trn_rl_repo',):
    if _p not in sys.path:
        sys.path.insert(0, _p)

from contextlib import ExitStack

import numpy as np

import concourse.bass as bass
import concourse.bacc as bacc
import concourse.mybir as mybir
import concourse.tile as tile
from concourse import bass_utils

# Note: walrus's LDWEIGHTS optimizer (--enable-ldw-opt=true) crashes codegen
# on this kernel (visitInstLdweights INTERNAL_ERROR), so the stock
# flags stay as-is.

F32 = mybir.dt.float32
F32R = mybir.dt.float32r
BF16 = mybir.dt.bfloat16
AF = mybir.ActivationFunctionType

B, N, D, H, HD = 8, 1024, 768, 12, 64
P = 128
DT = D // P            # 6 d-tiles
PAIRS = H // 2         # 6 head pairs (== DT)
SCALE = HD ** -0.5
NEGMASK = -30000.0     # exp(x + NEGMASK) == 0.0 in fp32 for any realistic score
BIGG = 1e15            # diagonal dominance constant for padded-query rows


def build_nc(n=N, debug=False):
    NT = n // P                    # seq tiles (8)
    nc = bacc.Bacc("TRN2", target_bir_lowering=False, debug=False)

    # xT is packed t-major ([P, t, d, 128]) and wqkT pair-major
    # ([P, pair, q/k, d, 128]) so every DMA chunk is contiguous per partition.
    xT_d = nc.dram_tensor("xT", [P, DT * n], F32, kind="ExternalInput")
    wqkT_d = nc.dram_tensor("wqkT", [P, DT * 2 * D], F32, kind="ExternalInput")
    wvT_d = nc.dram_tensor("wvT", [P, DT * D], F32, kind="ExternalInput")
    wprojT_d = nc.dram_tensor("wprojT", [P, DT * D], F32, kind="ExternalInput")
    bproj_d = nc.dram_tensor("bproj", [1, D], F32, kind="ExternalInput")
    mbias_d = nc.dram_tensor("mbias", [P, NT], F32, kind="ExternalInput")
    omm_d = nc.dram_tensor("omm", [P, NT], F32, kind="ExternalInput")
    ones_d = nc.dram_tensor("onesv", [1, 512], F32, kind="ExternalInput")
    out_d = nc.dram_tensor("out", [D, n], F32, kind="ExternalOutput")

    def rr(ap):
        return ap.bitcast(F32R)

    with tile.TileContext(nc) as tc, ExitStack() as ctx:
        persist = ctx.enter_context(tc.tile_pool(name="persist", bufs=1))
        xt = persist.tile([P, NT, DT, P], F32R)        # 24KB/part, t-major
        wq = persist.tile([P, PAIRS, 2, DT, P], F32R)  # 36KB, pair-major
        wpj = persist.tile([P, DT, D], F32R)           # 18KB
        vaug = persist.tile([P, NT, PAIRS, 2 * P], BF16)  # 24KB
        qks = persist.tile([P, 3, 2, n], BF16)         # 12KB (pair slots: q,k)
        otn = persist.tile([P, DT, n], F32R)           # 24KB
        pa = persist.tile([P, NT, n], BF16)            # 16KB
        pb = persist.tile([P, NT, n], BF16)            # 16KB
        dtl = persist.tile([P, NT, P], BF16)           # 2KB
        mb = persist.tile([P, NT], F32)
        om = persist.tile([P, NT], F32)
        ones = persist.tile([1, 512], F32R)
        bpj = persist.tile([1, D], F32R)

        # ---------------- input DMAs, need-ordered across 3 queues ---------
        xt_src = rr(xT_d.ap()).rearrange("p (t d c) -> p t d c", t=NT, d=DT)
        wqk_src = rr(wqkT_d.ap()).rearrange("p (pr j d c) -> p pr j d c",
                                            pr=PAIRS, j=2, d=DT)
        wv_src = rr(wvT_d.ap()).rearrange("p (dt e) -> p dt e", dt=DT)
        wpj_src = rr(wprojT_d.ap()).rearrange("p (dt e) -> p dt e", dt=DT)

        # gpsimd queue: small tensors, then the (late-needed) proj weights
        nc.gpsimd.dma_start(mb, mbias_d.ap())
        nc.gpsimd.dma_start(om, omm_d.ap())
        nc.gpsimd.dma_start(ones, rr(ones_d.ap()))
        nc.gpsimd.dma_start(bpj, rr(bproj_d.ap()))
        nc.gpsimd.dma_start(wpj, wpj_src)

        # sync queue: x by seq-tile (v-phase consumes in t order), then
        # q/k weights for pairs 2..5 (filler work, needed progressively)
        for t in range(NT):
            nc.sync.dma_start(xt[:, t], xt_src[:, t])
        for pr in range(2, PAIRS):
            nc.sync.dma_start(wq[:, pr], wqk_src[:, pr])

        # scalar queue: v weights (needed first), then q/k for pairs 0,1
        with tc.tile_pool(name="wvpool", bufs=1) as wvp, \
             tc.tile_pool(name="scratch", bufs=1) as scr:
            wv = wvp.tile([P, DT, D], F32R)            # 18KB, freed after v
            dtf = scr.tile([P, NT, P], F32)            # 4KB, prologue only
            warm = scr.tile([1, 1], F32)
            # fine-grained wv chunks so v-phase d-loop starts ASAP
            for d in range(DT):
                nc.scalar.dma_start(wv[:, d, 0:512], wv_src[:, d, 0:512])
            for d in range(DT):
                nc.scalar.dma_start(wv[:, d, 512:D], wv_src[:, d, 512:D])
            for pr in range(2):
                nc.scalar.dma_start(wq[:, pr], wqk_src[:, pr])

            # prologue compute: ones blocks of vaug, diag tiles, ACT warmup
            nc.vector.memset(vaug[:, :, :, HD:3 * HD], 1.0)
            for t in range(NT):
                nc.gpsimd.affine_select(
                    out=dtf[:, t, :],
                    in_=om[:, t:t + 1].to_broadcast((P, P)),
                    pattern=[[-1, P]],
                    compare_op=mybir.AluOpType.is_equal,
                    fill=0.0, base=0, channel_multiplier=1,
                )
            nc.vector.tensor_copy(dtl, dtf)
            # load the exp table set during the v-phase, not at first score
            nc.scalar.activation(warm, mb[0:1, 0:1], AF.Exp, bias=0.0, scale=1.0)

            with tc.tile_pool(name="stps", bufs=2, space="PSUM") as stps, \
                 tc.tile_pool(name="otps", bufs=2, space="PSUM") as otps, \
                 tc.tile_pool(name="rbsp", bufs=2) as rbsp:

                # ---------------- v-phase ----------------
                # per t: 12 accumulating MMs into one [P, 768] PSUM region,
                # then strided copies into vaug's pair blocks (even heads at
                # block cols 0:64, odd heads at 192:256).
                def emit_v(t):
                    vp = otps.tile([P, n], F32, tag="ot", name="vp")
                    for (cb, cw) in ((0, 512), (512, 256)):
                        for d in range(DT):
                            nc.tensor.matmul(
                                vp[:, cb:cb + cw],
                                xt[:, t, d, :],
                                wv[:, d, cb:cb + cw],
                                start=(d == 0), stop=(d == DT - 1))
                    vsp = vp[:, 0:D].rearrange("p (pr h d) -> p pr h d",
                                               h=2, d=HD)
                    # evens on ScE, odds on DVE (both engines idle here)
                    nc.scalar.copy(vaug[:, t, :, 0:HD], vsp[:, :, 0, :])
                    nc.vector.tensor_copy(vaug[:, t, :, 3 * HD:4 * HD],
                                          vsp[:, :, 1, :])

                # ---------------- qk units + attention pipeline ----------
                def emit_qk_quarter(pr, j, c):
                    """One c-half of q (j=0) or k (j=1) for pair pr: 6 MMs."""
                    key = (pr, j)
                    if key not in qk_ps:
                        qk_ps[key] = stps.tile([P, n], F32, tag="st",
                                               name="qkp")
                    ps = qk_ps[key]
                    for d in range(DT):
                        nc.tensor.matmul(
                            ps[:, c * 512:(c + 1) * 512],
                            wq[:, pr, j, d, :],
                            xt[:, 4 * c:4 * c + 4, d, :],
                            start=(d == 0), stop=(d == DT - 1))
                    if c == 1:
                        nc.vector.tensor_copy(qks[:, pr % 3, j, :], ps)
                        del qk_ps[key]

                # pre-phase schedule: v tiles interleaved with qk(0); the
                # c=0 qk quarters only need x t0-3, so they fill the PE
                # while x t4-7 is still streaming in.
                qk_ps = {}
                for t in range(4):
                    emit_v(t)
                emit_qk_quarter(0, 0, 0)
                emit_qk_quarter(0, 1, 0)
                for t in range(4, NT):
                    emit_v(t)
                emit_qk_quarter(0, 0, 1)
                emit_qk_quarter(0, 1, 1)

                ot_tiles = {}

                def emit_st(pr, hi, t):
                    slot = pr % 3
                    lo = hi * HD
                    st = stps.tile([P, n], F32, tag="st", name="st")
                    for c in range(2):
                        sl = slice(c * 512, (c + 1) * 512)
                        nc.tensor.matmul(
                            st[:, sl],
                            qks[lo:lo + HD, slot, 1, t * P:(t + 1) * P],
                            qks[lo:lo + HD, slot, 0, sl],
                            start=True, stop=True)
                    px = pa if hi == 0 else pb
                    nc.scalar.activation(px[:, t, :], st, AF.Exp,
                                         bias=mb[:, t:t + 1], scale=SCALE)
                    nc.vector.tensor_add(px[:, t, t * P:(t + 1) * P],
                                         px[:, t, t * P:(t + 1) * P],
                                         dtl[:, t, :])

                def emit_pv(pr, hi, t):
                    if (pr, hi) not in ot_tiles:
                        ot_tiles[(pr, hi)] = otps.tile([P, n], F32, tag="ot",
                                                       name="ot")
                    ot = ot_tiles[(pr, hi)]
                    px = pa if hi == 0 else pb
                    for c in range(2):
                        sl = slice(c * 512, (c + 1) * 512)
                        nc.tensor.matmul(
                            ot[:, sl], vaug[:, t, pr, hi * P:(hi + 1) * P],
                            px[:, t, sl],
                            start=(t == 0), stop=(t == NT - 1),
                            skip_group_check=True)

                def emit_norm(pr, hi):
                    # reciprocal_approx_fast only works at partition base 0
                    # (custom-DVE op); stage Z to base 0 where needed.
                    ot = ot_tiles.pop((pr, hi))
                    if hi == 0:      # PV on 0:64, Z replicated on 64:128
                        zst = rbsp.tile([P, n], F32, tag="rbs", name="zst")
                        rbs = rbsp.tile([P, n], F32, tag="rbs", name="rbs")
                        nc.vector.tensor_copy(zst[HD:P, :], ot[HD:P, :])
                        nc.gpsimd.dma_start(zst[0:HD, :], zst[HD:P, :])
                        nc.vector.reciprocal_approx_fast(rbs[0:HD, :],
                                                         zst[0:HD, :])
                        nc.vector.tensor_mul(otn[0:HD, pr, :], ot[0:HD, :],
                                             rbs[0:HD, :])
                    else:            # Z replicated on 0:64, PV on 64:128
                        rbs = rbsp.tile([P, n], F32, tag="rbs", name="rbs")
                        nc.vector.reciprocal_approx_fast(rbs[0:HD, :],
                                                         ot[0:HD, :])
                        nc.gpsimd.dma_start(rbs[HD:P, :], rbs[0:HD, :])
                        nc.vector.tensor_mul(otn[HD:P, pr, :], ot[HD:P, :],
                                             rbs[HD:P, :])

                groups = [(pr, hi, t)
                          for pr in range(PAIRS)
                          for hi in range(2) for t in range(NT)]
                LAG = 2
                for i, g in enumerate(groups):
                    emit_st(*g)
                    pr, hi, t = g
                    # filler: qk projection for pair pr+1 (absorbs PE slack;
                    # keeps HAM warm).  Quarter-units at adjacent groups so a
                    # held accumulation spans at most one 'st' ring slot.
                    if pr + 1 < PAIRS and t in (2, 3):
                        emit_qk_quarter(pr + 1, hi, t - 2)
                    if i >= LAG:
                        gj = groups[i - LAG]
                        emit_pv(*gj)
                        if gj[2] == NT - 1:
                            emit_norm(gj[0], gj[1])
                for i in range(len(groups) - LAG, len(groups)):
                    gj = groups[i]
                    emit_pv(*gj)
                    if gj[2] == NT - 1:
                        emit_norm(gj[0], gj[1])
                # bridge matmuls: keep the PE (and its HAM clock) busy while
                # the last norm chain runs, so the projection starts warm.
                # Results are never read.
                for c in range(2):
                    brg = stps.tile([P, n], F32, tag="st", name="brg_v6")
                    for d in range(DT):
                        nc.tensor.matmul(
                            brg[:, c * 512:(c + 1) * 512],
                            wq[:, 0, 0, d, :], xt[:, 4 * c:4 * c + 4, d, :],
                            start=(d == 0), stop=(d == DT - 1))

        # ---------------- output projection (transposed: outT[e, n]) -------
        # wproj blocks are the stationary operand (6x fewer LDWEIGHTS than
        # otn-stationary); the host transposes the [D, n] result back.
        with tc.tile_pool(name="pjp", bufs=3, space="PSUM") as pjp, \
             tc.tile_pool(name="obp", bufs=3) as obp:
            for et in range(DT):
                ps = pjp.tile([P, n], F32, tag="pj")
                ob = obp.tile([P, n], F32, tag="ob")
                for c in range(2):
                    sl = slice(c * 512, (c + 1) * 512)
                    # bias first: it has no dependency on the attention
                    # output, so it issues during the final norm chain
                    nc.tensor.matmul(ps[:, sl],
                                     bpj[:, et * P:(et + 1) * P],
                                     ones, start=True, stop=False)
                    for d in range(DT):
                        nc.tensor.matmul(ps[:, sl],
                                         wpj[:, d, et * P:(et + 1) * P],
                                         otn[:, d, sl],
                                         start=False, stop=(d == DT - 1))
                # ACT and DVE are both idle post-attention: split the drain
                nc.scalar.copy(ob[:, 0:512], ps[:, 0:512])
                nc.vector.tensor_copy(ob[:, 512:n], ps[:, 512:n])
                (nc.sync if et % 2 == 0 else nc.scalar).dma_start(
                    out_d.ap()[et * P:(et + 1) * P, :], ob)

    nc.compile()
    return nc


def make_in_maps(x, mask, Wqkv, Wproj, bproj):
    x = np.ascontiguousarray(np.asarray(x), dtype=np.float32)
    mask = np.asarray(mask)
    def pack(wt):   # [D, cols] -> [128, DT*cols], row p = concat_d wt[d*128+p]
        cols = wt.shape[1]
        return np.ascontiguousarray(
            wt.reshape(DT, P, cols).transpose(1, 0, 2).reshape(P, DT * cols))
    wqkvT = np.asarray(Wqkv, dtype=np.float32).T.copy()   # [D, 3D]
    # pair-major q/k pack: [P, pair, q/k, d, 128]
    wqkT = np.ascontiguousarray(
        wqkvT[:, 0:2 * D].reshape(DT, P, 2, PAIRS, P)
        .transpose(1, 3, 2, 0, 4).reshape(P, DT * 2 * D))
    wvT = pack(wqkvT[:, 2 * D:3 * D])
    wprojT = pack(np.asarray(Wproj, dtype=np.float32).T.copy())
    bp = np.ascontiguousarray(np.asarray(bproj, dtype=np.float32).reshape(1, D))
    onesv = np.ones((1, 512), dtype=np.float32)
    b, n, _ = x.shape
    nt = n // P
    def pack_x_tmajor(xTi):   # [D, n] -> [P, t, d, 128] flattened
        return np.ascontiguousarray(
            xTi.reshape(DT, P, nt, P).transpose(1, 2, 0, 3)
            .reshape(P, DT * n))
    in_maps = []
    for i in range(b):
        mf = mask[i].astype(np.float32)
        mcol = mf.reshape(nt, P).T.copy()              # [P, NT]
        in_maps.append({
            "xT": pack_x_tmajor(np.ascontiguousarray(x[i].T)),
            "wqkT": wqkT,
            "wvT": wvT,
            "wprojT": wprojT,
            "bproj": bp,
            "mbias": np.ascontiguousarray((mcol - 1.0) * (-NEGMASK)),
            "omm": np.ascontiguousarray((1.0 - mcol) * BIGG),
            "onesv": onesv,
        })
    return in_maps


_NC_CACHE = {}


def get_nc(n=N):
    if n not in _NC_CACHE:
        _NC_CACHE[n] = build_nc(n)
    return _NC_CACHE[n]


def kernel(x, mask, Wqkv, Wproj, bproj):
    x = np.asarray(x)
    b, n, _ = x.shape
    nc = get_nc(n)
    in_maps = make_in_maps(x, mask, Wqkv, Wproj, bproj)
    res = bass_utils.run_bass_kernel_spmd(nc, in_maps, core_ids=list(range(b)))
    out = np.stack([np.asarray(res.results[i]["out"]).T for i in range(b)],
                   axis=0)
    return np.ascontiguousarray(out).astype(np.float32)


# revision 29
# speedup vs baseline: 1.1208x; 1.0712x over previous
"""Masked multi-head attention block (B=8, N=1024, D=768, H=12) on 8 NeuronCores.

Strategy: pure data-parallel over batch (1 batch element per core).  Per core,
one fully-fused software pipeline keeps the PE array busy (and its HAM clock
gate warm) end to end:

  prologue:  need-ordered chunked input DMA across the three DMA queues;
             first matmul issues ~6us in.
  v-phase:   v[n, e] = xT(lhsT) @ WvT, stored bf16 in per-head-pair blocks
             [v_even | ones64][ones64 | v_odd]: the 64 ones columns replicate
             the softmax denominator Z across 64 PSUM partitions for free.
  qk(pr):    q,k e-tiles of head-pair pr (f32r matmuls, cast to bf16).
             Pairs 0,1 run before the attention loop; pair pr+2 is emitted as
             PE *filler* inside attention pair pr, exactly absorbing the PE
             slack of the ACT-bound softmax loop (keeps HAM at 2.4 GHz).
  attention: per (pair, hi, t): ST = kT(lhsT) @ qT (bf16, 1024-wide moving),
             P = exp(ST*scale + key_mask_bias) on ACT (PSUM -> SBUF bf16),
             diag add (padded queries -> one-hot), P@Vaug accumulated over t
             (PV on one 64-partition half, Z replicated on the other), then
             norm: recip(Z-half) -> tiny partition-shift DMA -> mul into otn.
  proj:      out[n, e] = otn(lhsT) @ WprojT + ones(K=1) x bproj, accumulated
             in PSUM and DMA'd straight PSUM -> DRAM (no SBUF staging).

Numerics: QKV/proj contractions (K=768) in float32r; q/k/v/P in bf16 (errors
~1e-3 absolute vs a ~6.0 absmax reference; tolerance is 2e-2 relative).
"""
import sys
for _p in ('/opt/trn_rl_repo',):
    if _p not in sys.path:
        sys.path.insert(0, _p)

from contextlib import ExitStack

import numpy as np

import concourse.bass as bass
import concourse.bacc as bacc
import concourse.mybir as mybir
import concourse.tile as tile
from concourse import bass_utils

# Note: walrus's LDWEIGHTS optimizer (--enable-ldw-opt=true) crashes codegen
# on this kernel (visitInstLdweights INTERNAL_ERROR), so the stock
# flags stay as-is.

F32 = mybir.dt.float32
F32R = mybir.dt.float32r
BF16 = mybir.dt.bfloat16
AF = mybir.ActivationFunctionType

B, N, D, H, HD = 8, 1024, 768, 12, 64
P = 128
DT = D // P            # 6 d-tiles
PAIRS = H // 2         # 6 head pairs (== DT)
SCALE = HD ** -0.5
NEGMASK = -30000.0     # exp(x + NEGMASK) == 0.0 in fp32 for any realistic score
BIGG = 1e15            # diagonal dominance constant for padded-query rows


def build_nc(n=N, debug=False):
    NT = n // P                    # seq tiles (8)
    nc = bacc.Bacc("TRN2", target_bir_lowering=False, debug=False)

    # xT is packed t-major ([P, t, d, 128]) and wqkT pair-major
    # ([P, pair, q/k, d, 128]) so every DMA chunk is contiguous per partition.
    xT_d = nc.dram_tensor("xT", [P, DT * n], F32, kind="ExternalInput")
    wqkT_d = nc.dram_tensor("wqkT", [P, DT * 2 * D], F32, kind="ExternalInput")
    wvT_d = nc.dram_tensor("wvT", [P, DT * D], F32, kind="ExternalInput")
    wprojT_d = nc.dram_tensor("wprojT", [P, DT * D], F32, kind="ExternalInput")
    bproj_d = nc.dram_tensor("bproj", [1, D], F32, kind="ExternalInput")
    mbias_d = nc.dram_tensor("mbias", [P, NT], F32, kind="ExternalInput")
    omm_d = nc.dram_tensor("omm", [P, NT], F32, kind="ExternalInput")
    ones_d = nc.dram_tensor("onesv", [1, 512], F32, kind="ExternalInput")
    out_d = nc.dram_tensor("out", [D, n], F32, kind="ExternalOutput")

    def rr(ap):
        return ap.bitcast(F32R)

    with tile.TileContext(nc) as tc, ExitStack() as ctx:
        persist = ctx.enter_context(tc.tile_pool(name="persist", bufs=1))
        xt = persist.tile([P, NT, DT, P], F32R)        # 24KB/part, t-major
        wq = persist.tile([P, PAIRS, 2, DT, P], F32R)  # 36KB, pair-major
        wpj = persist.tile([P, DT, D], F32R)           # 18KB
        vaug = persist.tile([P, NT, PAIRS, 2 * P], BF16)  # 24KB
        qks = persist.tile([P, 3, 2, n], BF16)         # 12KB (pair slots: q,k)
        otn = persist.tile([P, DT, n], F32R)           # 24KB
        pa = persist.tile([P, NT, n], BF16)            # 16KB
        pb = persist.tile([P, NT, n], BF16)            # 16KB
        dtl = persist.tile([P, NT, P], BF16)           # 2KB
        mb = persist.tile([P, NT], F32)
        om = persist.tile([P, NT], F32)
        ones = persist.tile([1, 512], F32R)
        bpj = persist.tile([1, D], F32R)

        # ---------------- input DMAs, need-ordered across 3 queues ---------
        xt_src = rr(xT_d.ap()).rearrange("p (t d c) -> p t d c", t=NT, d=DT)
        wqk_src = rr(wqkT_d.ap()).rearrange("p (pr j d c) -> p pr j d c",
                                            pr=PAIRS, j=2, d=DT)
        wv_src = rr(wvT_d.ap()).rearrange("p (dt e) -> p dt e", dt=DT)
        wpj_src = rr(wprojT_d.ap()).rearrange("p (dt e) -> p dt e", dt=DT)

        # gpsimd queue: small tensors only (keeps its software DGE free for
        # the norm partition-shift hops during attention)
        nc.gpsimd.dma_start(mb, mbias_d.ap())
        nc.gpsimd.dma_start(om, omm_d.ap())
        nc.gpsimd.dma_start(ones, rr(ones_d.ap()))
        nc.gpsimd.dma_start(bpj, rr(bproj_d.ap()))

        # sync queue: x by seq-tile (v-phase consumes in t order), then
        # q/k weights for pairs 2..5 (filler work, needed progressively),
        # and the proj weights last (needed only at the epilogue) so they
        # don't steal early bandwidth from x.
        for t in range(NT):
            nc.sync.dma_start(xt[:, t], xt_src[:, t])
        for pr in range(2, PAIRS):
            nc.sync.dma_start(wq[:, pr], wqk_src[:, pr])
        nc.sync.dma_start(wpj, wpj_src)

        # scalar queue: v weights (needed first), then q/k for pairs 0,1
        with tc.tile_pool(name="wvpool", bufs=1) as wvp, \
             tc.tile_pool(name="scratch", bufs=1) as scr:
            wv = wvp.tile([P, DT, D], F32R)            # 18KB, freed after v
            dtf = scr.tile([P, NT, P], F32)            # 4KB, prologue only
            warm = scr.tile([1, 1], F32)
            # fine-grained wv chunks so v-phase d-loop starts ASAP
            for d in range(DT):
                nc.scalar.dma_start(wv[:, d, 0:512], wv_src[:, d, 0:512])
            for d in range(DT):
                nc.scalar.dma_start(wv[:, d, 512:D], wv_src[:, d, 512:D])
            for pr in range(2):
                nc.scalar.dma_start(wq[:, pr], wqk_src[:, pr])

            # prologue compute: ones blocks of vaug, diag tiles, ACT warmup
            nc.vector.memset(vaug[:, :, :, HD:3 * HD], 1.0)
            for t in range(NT):
                nc.gpsimd.affine_select(
                    out=dtf[:, t, :],
                    in_=om[:, t:t + 1].to_broadcast((P, P)),
                    pattern=[[-1, P]],
                    compare_op=mybir.AluOpType.is_equal,
                    fill=0.0, base=0, channel_multiplier=1,
                )
            nc.vector.tensor_copy(dtl, dtf)
            # load the exp table set during the v-phase, not at first score
            nc.scalar.activation(warm, mb[0:1, 0:1], AF.Exp, bias=0.0, scale=1.0)

            with tc.tile_pool(name="stps", bufs=2, space="PSUM") as stps, \
                 tc.tile_pool(name="otps", bufs=2, space="PSUM") as otps, \
                 tc.tile_pool(name="rbsp", bufs=2) as rbsp:

                # ---------------- v-phase ----------------
                # per t: 12 accumulating MMs into one [P, 768] PSUM region,
                # then strided copies into vaug's pair blocks (even heads at
                # block cols 0:64, odd heads at 192:256).
                def emit_v(t):
                    vp = otps.tile([P, n], F32, tag="ot", name="vp")
                    for (cb, cw) in ((0, 512), (512, 256)):
                        for d in range(DT):
                            nc.tensor.matmul(
                                vp[:, cb:cb + cw],
                                xt[:, t, d, :],
                                wv[:, d, cb:cb + cw],
                                start=(d == 0), stop=(d == DT - 1))
                    vsp = vp[:, 0:D].rearrange("p (pr h d) -> p pr h d",
                                               h=2, d=HD)
                    # evens on ScE, odds on DVE (both engines idle here)
                    nc.scalar.copy(vaug[:, t, :, 0:HD], vsp[:, :, 0, :])
                    nc.vector.tensor_copy(vaug[:, t, :, 3 * HD:4 * HD],
                                          vsp[:, :, 1, :])

                # ---------------- qk units + attention pipeline ----------
                def emit_qk_quarter(pr, j, c):
                    """One c-half of q (j=0) or k (j=1) for pair pr: 6 MMs."""
                    key = (pr, j)
                    if key not in qk_ps:
                        qk_ps[key] = stps.tile([P, n], F32, tag="st",
                                               name="qkp")
                    ps = qk_ps[key]
                    for d in range(DT):
                        nc.tensor.matmul(
                            ps[:, c * 512:(c + 1) * 512],
                            wq[:, pr, j, d, :],
                            xt[:, 4 * c:4 * c + 4, d, :],
                            start=(d == 0), stop=(d == DT - 1))
                    if c == 1:
                        nc.vector.tensor_copy(qks[:, pr % 3, j, :], ps)
                        del qk_ps[key]

                # pre-phase schedule: v tiles interleaved with qk(0); the
                # c=0 qk quarters only need x t0-3, so they fill the PE
                # while x t4-7 is still streaming in.
                qk_ps = {}
                for t in range(4):
                    emit_v(t)
                emit_qk_quarter(0, 0, 0)
                emit_qk_quarter(0, 1, 0)
                for t in range(4, NT):
                    emit_v(t)
                emit_qk_quarter(0, 0, 1)
                emit_qk_quarter(0, 1, 1)

                ot_tiles = {}

                def emit_st(pr, hi, t):
                    slot = pr % 3
                    lo = hi * HD
                    st = stps.tile([P, n], F32, tag="st", name="st")
                    for c in range(2):
                        sl = slice(c * 512, (c + 1) * 512)
                        nc.tensor.matmul(
                            st[:, sl],
                            qks[lo:lo + HD, slot, 1, t * P:(t + 1) * P],
                            qks[lo:lo + HD, slot, 0, sl],
                            start=True, stop=True)
                    px = pa if hi == 0 else pb
                    nc.scalar.activation(px[:, t, :], st, AF.Exp,
                                         bias=mb[:, t:t + 1], scale=SCALE)
                    nc.vector.tensor_add(px[:, t, t * P:(t + 1) * P],
                                         px[:, t, t * P:(t + 1) * P],
                                         dtl[:, t, :])

                def emit_pv(pr, hi, t):
                    if (pr, hi) not in ot_tiles:
                        ot_tiles[(pr, hi)] = otps.tile([P, n], F32, tag="ot",
                                                       name="ot")
                    ot = ot_tiles[(pr, hi)]
                    px = pa if hi == 0 else pb
                    for c in range(2):
                        sl = slice(c * 512, (c + 1) * 512)
                        nc.tensor.matmul(
                            ot[:, sl], vaug[:, t, pr, hi * P:(hi + 1) * P],
                            px[:, t, sl],
                            start=(t == 0), stop=(t == NT - 1),
                            skip_group_check=True)

                def emit_norm(pr, hi):
                    # reciprocal_approx_fast only works at partition base 0
                    # (custom-DVE op); stage Z to base 0 where needed.
                    ot = ot_tiles.pop((pr, hi))
                    if hi == 0:      # PV on 0:64, Z replicated on 64:128
                        zst = rbsp.tile([P, n], F32, tag="rbs", name="zst")
                        rbs = rbsp.tile([P, n], F32, tag="rbs", name="rbs")
                        nc.vector.tensor_copy(zst[HD:P, :], ot[HD:P, :])
                        nc.gpsimd.dma_start(zst[0:HD, :], zst[HD:P, :])
                        nc.vector.reciprocal_approx_fast(rbs[0:HD, :],
                                                         zst[0:HD, :])
                        nc.vector.tensor_mul(otn[0:HD, pr, :], ot[0:HD, :],
                                             rbs[0:HD, :])
                    else:            # Z replicated on 0:64, PV on 64:128
                        rbs = rbsp.tile([P, n], F32, tag="rbs", name="rbs")
                        nc.vector.reciprocal_approx_fast(rbs[0:HD, :],
                                                         ot[0:HD, :])
                        nc.gpsimd.dma_start(rbs[HD:P, :], rbs[0:HD, :])
                        nc.vector.tensor_mul(otn[HD:P, pr, :], ot[HD:P, :],
                                             rbs[HD:P, :])

                groups = [(pr, hi, t)
                          for pr in range(PAIRS)
                          for hi in range(2) for t in range(NT)]
                LAG = 2
                for i, g in enumerate(groups):
                    emit_st(*g)
                    pr, hi, t = g
                    # filler: qk projection for pair pr+1 (absorbs PE slack;
                    # keeps HAM warm).  Quarter-units at adjacent groups so a
                    # held accumulation spans at most one 'st' ring slot.
                    if pr + 1 < PAIRS and t in (2, 3):
                        emit_qk_quarter(pr + 1, hi, t - 2)
                    if i >= LAG:
                        gj = groups[i - LAG]
                        emit_pv(*gj)
                        if gj[2] == NT - 1:
                            emit_norm(gj[0], gj[1])
                for i in range(len(groups) - LAG, len(groups)):
                    gj = groups[i]
                    emit_pv(*gj)
                    if gj[2] == NT - 1:
                        emit_norm(gj[0], gj[1])
                # bridge matmuls: keep the PE (and its HAM clock) busy while
                # the last norm chain runs, so the projection starts warm.
                # Results are never read.
                for u in range(4):
                    brg = stps.tile([P, n], F32, tag="st", name="brg_v6")
                    for c in range(2):
                        for d in range(DT):
                            nc.tensor.matmul(
                                brg[:, c * 512:(c + 1) * 512],
                                wq[:, u % 2, u // 2, d, :],
                                xt[:, 4 * c:4 * c + 4, d, :],
                                start=(d == 0), stop=(d == DT - 1))

        # ---------------- output projection (transposed: outT[e, n]) -------
        # wproj blocks are the stationary operand (6x fewer LDWEIGHTS than
        # otn-stationary); the host transposes the [D, n] result back.
        with tc.tile_pool(name="pjp", bufs=3, space="PSUM") as pjp, \
             tc.tile_pool(name="obp", bufs=3) as obp:
            for et in range(DT):
                ps = pjp.tile([P, n], F32, tag="pj")
                ob = obp.tile([P, n], F32, tag="ob")
                for c in range(2):
                    sl = slice(c * 512, (c + 1) * 512)
                    # bias first: it has no dependency on the attention
                    # output, so it issues during the final norm chain
                    nc.tensor.matmul(ps[:, sl],
                                     bpj[:, et * P:(et + 1) * P],
                                     ones, start=True, stop=False)
                    for d in range(DT):
                        nc.tensor.matmul(ps[:, sl],
                                         wpj[:, d, et * P:(et + 1) * P],
                                         otn[:, d, sl],
                                         start=False, stop=(d == DT - 1))
                # ACT and DVE are both idle post-attention: split the drain
                nc.scalar.copy(ob[:, 0:512], ps[:, 0:512])
                nc.vector.tensor_copy(ob[:, 512:n], ps[:, 512:n])
                (nc.sync if et % 2 == 0 else nc.scalar).dma_start(
                    out_d.ap()[et * P:(et + 1) * P, :], ob)

    nc.compile()
    return nc


def make_in_maps(x, mask, Wqkv, Wproj, bproj):
    x = np.ascontiguousarray(np.asarray(x), dtype=np.float32)
    mask = np.asarray(mask)
    def pack(wt):   # [D, cols] -> [128, DT*cols], row p = concat_d wt[d*128+p]
        cols = wt.shape[1]
        return np.ascontiguousarray(
            wt.reshape(DT, P, cols).transpose(1, 0, 2).reshape(P, DT * cols))
    wqkvT = np.asarray(Wqkv, dtype=np.float32).T.copy()   # [D, 3D]
    # pair-major q/k pack: [P, pair, q/k, d, 128]
    wqkT = np.ascontiguousarray(
        wqkvT[:, 0:2 * D].reshape(DT, P, 2, PAIRS, P)
        .transpose(1, 3, 2, 0, 4).reshape(P, DT * 2 * D))
    wvT = pack(wqkvT[:, 2 * D:3 * D])
    wprojT = pack(np.asarray(Wproj, dtype=np.float32).T.copy())
    bp = np.ascontiguousarray(np.asarray(bproj, dtype=np.float32).reshape(1, D))
    onesv = np.ones((1, 512), dtype=np.float32)
    b, n, _ = x.shape
    nt = n // P
    def pack_x_tmajor(xTi):   # [D, n] -> [P, t, d, 128] flattened
        return np.ascontiguousarray(
            xTi.reshape(DT, P, nt, P).transpose(1, 2, 0, 3)
            .reshape(P, DT * n))
    in_maps = []
    for i in range(b):
        mf = mask[i].astype(np.float32)
        mcol = mf.reshape(nt, P).T.copy()              # [P, NT]
        in_maps.append({
            "xT": pack_x_tmajor(np.ascontiguousarray(x[i].T)),
            "wqkT": wqkT,
            "wvT": wvT,
            "wprojT": wprojT,
            "bproj": bp,
            "mbias": np.ascontiguousarray((mcol - 1.0) * (-NEGMASK)),
            "omm": np.ascontiguousarray((1.0 - mcol) * BIGG),
            "onesv": onesv,
        })
    return in_maps


_NC_CACHE = {}


def get_nc(n=N):
    if n not in _NC_CACHE:
        _NC_CACHE[n] = build_nc(n)
    return _NC_CACHE[n]


def kernel(x, mask, Wqkv, Wproj, bproj):
    x = np.asarray(x)
    b, n, _ = x.shape
    nc = get_nc(n)
    in_maps = make_in_maps(x, mask, Wqkv, Wproj, bproj)
    res = bass_utils.run_bass_kernel_spmd(nc, in_maps, core_ids=list(range(b)))
    out = np.stack([np.asarray(res.results[i]["out"]).T for i in range(b)],
                   axis=0)
    return np.ascontiguousarray(out).astype(np.float32)
